# revision 1
# baseline (speedup 1.0000x reference)
"""nn_ContactHead Trainium2 kernel (8-core data parallel).

out = sigmoid(w2 . relu((grid_sample(feat, uv) @ reduce_w + reduce_b) @ cls_w1 + cls_b1) + cls_b2)

Everything left of the relu is linear and bilinear sampling is linear in the
features, so the channel reductions commute with the sampling:
  W  = reduce_w @ cls_w1            (1280 x 128)   [device, PE]
  bb = reduce_b @ cls_w1 + cls_b1   (128)          [device, PE via ones-row]
  z[d, pix] = feat[:, pix].W[:, d] + bb[d]    at the 1024 pixels (PE, bf16)
Bilinear via pre-differenced pixel quantities (one gather row per vert):
  dzx = z(x+1)-z ; dzy = z(y+1)-z ; dzxy = dzy(x+1)-dzy
  v(wx,wy) = z00 + wx*dzx + wy*(dzy + wx*dzxy)
Tokens [z00|dzx|dzy|dzxy] (1KB bf16 rows, pixel-major) are written to DRAM
(PE transpose), then fetched per-vert with the hardware DMA gather
(non-transpose => verts land on partitions, 128 dims x 4 quantities on free).
Blend on DVE with free-dim step-0 broadcast weight APs, relu+w2 fused via
scalar_tensor_tensor, dot via tensor_reduce, sigmoid on ACT.

Vert layout: vert j lives at (partition j%128, column j//128).
"""

import ml_dtypes
import numpy as np

B, C, H, W, N = 32, 1280, 32, 32, 6890
NCORES = 8
IMGS = B // NCORES          # 4 images per core
PIX = H * W                 # 1024
PPAD = 1088                 # padded pixel slots in the dims-major z tiles
NCH = C // 128              # 10 channel chunks
MID = 128
NV = 6912                   # padded verts (= 54*128)
Q = NV // 128               # 54
VCH = 6                     # vert chunks per image
VCN = NV // VCH             # 1152 = 9*128
VROW = VCN // 128           # 9 rows per gathered chunk tile
TOK = 512                   # token row: 4 quantities x 128 dims (bf16)

_CACHE = {}


def _build():
    if "nc" in _CACHE:
        return _CACHE["nc"]

    from contextlib import ExitStack

    import concourse.bass as bass
    import concourse.tile as tile
    from concourse import bacc, mybir
    from concourse.ap import AP
    from concourse.bass import IndirectOffsetOnAxis

    f32 = mybir.dt.float32
    bf16 = mybir.dt.bfloat16
    i16 = mybir.dt.int16
    i32 = mybir.dt.int32
    OP = mybir.AluOpType
    ACT = mybir.ActivationFunctionType

    nc = bacc.Bacc("TRN2", target_bir_lowering=False, debug=False)

    feat_d = nc.dram_tensor("feat", [IMGS, C, PIX], bf16, kind="ExternalInput")
    uv_d = nc.dram_tensor("uv", [IMGS, NV, 2], f32, kind="ExternalInput")
    rwt_d = nc.dram_tensor("rwt", [256, C], f32, kind="ExternalInput")
    cw1_d = nc.dram_tensor("cw1", [256, MID], f32, kind="ExternalInput")
    rb_d = nc.dram_tensor("rb", [256], f32, kind="ExternalInput")
    cb1_d = nc.dram_tensor("cb1", [MID], f32, kind="ExternalInput")
    w2r_d = nc.dram_tensor("w2r", [128, 128], f32, kind="ExternalInput")
    cb2_d = nc.dram_tensor("cb2", [128, 1], f32, kind="ExternalInput")
    id_d = nc.dram_tensor("ident", [128, 128], bf16, kind="ExternalInput")
    ztok_d = [
        nc.dram_tensor(f"ztok{i}", [PIX, TOK], bf16) for i in range(IMGS)
    ]
    out_d = nc.dram_tensor("out", [IMGS, NV], f32, kind="ExternalOutput")

    with tile.TileContext(nc) as tc, ExitStack() as ctx:
        consts = ctx.enter_context(tc.tile_pool(name="consts", bufs=1))
        prep = ctx.enter_context(tc.tile_pool(name="prep", bufs=1))
        featp = ctx.enter_context(tc.tile_pool(name="featp", bufs=2))
        zqp = ctx.enter_context(tc.tile_pool(name="zqp", bufs=8))
        gpool = ctx.enter_context(tc.tile_pool(name="gpool", bufs=4))
        tpool = ctx.enter_context(tc.tile_pool(name="tpool", bufs=4))
        sm = ctx.enter_context(tc.tile_pool(name="sm", bufs=4))
        irp = ctx.enter_context(tc.tile_pool(name="irp", bufs=4))
        lg = ctx.enter_context(tc.tile_pool(name="lg", bufs=2))

        # ---------------- phase 0: combined weights (PE) ----------------
        psw_ctx = ExitStack()
        psw = psw_ctx.enter_context(tc.tile_pool(name="psw", bufs=2, space="PSUM"))
        rwt_t, cw1_t = [], []
        for k in range(2):
            rt = prep.tile([128, C], f32, tag=f"rwt{k}", name=f"rwt{k}")
            nc.sync.dma_start(rt[:], rwt_d.ap()[128 * k : 128 * (k + 1), :])
            rwt_t.append(rt)
            ct = prep.tile([128, MID], f32, tag=f"cw1{k}", name=f"cw1{k}")
            nc.sync.dma_start(ct[:], cw1_d.ap()[128 * k : 128 * (k + 1), :])
            cw1_t.append(ct)

        Wt = []
        for c in range(NCH):
            pw = psw.tile([128, 128], f32, tag="pw", name=f"pw{c}")
            for k in range(2):
                nc.tensor.matmul(
                    pw[:],
                    lhsT=rwt_t[k][:, 128 * c : 128 * (c + 1)],
                    rhs=cw1_t[k][:],
                    start=(k == 0),
                    stop=(k == 1),
                )
            wt = consts.tile([128, 128], bf16, tag=f"W{c}", name=f"W{c}")
            nc.scalar.copy(wt[:], pw[:])
            Wt.append(wt)

        rb_t = prep.tile([128, 2], f32, tag="rb", name="rb")
        nc.scalar.dma_start(rb_t[:], rb_d.ap().rearrange("(k p) -> p k", p=128))
        cb1_t = prep.tile([1, MID], f32, tag="cb1", name="cb1")
        nc.scalar.dma_start(cb1_t[:], cb1_d.ap().rearrange("(one d) -> one d", one=1))
        pb = psw.tile([1, 128], f32, tag="pb", name="pb")
        for k in range(2):
            nc.tensor.matmul(
                pb[:], lhsT=rb_t[:, k : k + 1], rhs=cw1_t[k][:],
                start=(k == 0), stop=(k == 1),
            )
        brow = prep.tile([1, 128], f32, tag="brow", name="brow")
        nc.vector.tensor_tensor(out=brow[:], in0=pb[:], in1=cb1_t[:], op=OP.add)
        bbias = consts.tile([1, 128], bf16, tag="bbias", name="bbias")
        nc.scalar.copy(bbias[:], brow[:])

        ones_t = consts.tile([1, PIX], bf16, tag="ones", name="ones")
        nc.vector.memset(ones_t[:], 1.0)
        ident = consts.tile([128, 128], bf16, tag="ident", name="ident")
        nc.scalar.dma_start(ident[:], id_d.ap())
        w2rf = prep.tile([128, 128], f32, tag="w2rf", name="w2rf")
        nc.scalar.dma_start(w2rf[:], w2r_d.ap())
        w2rep = consts.tile([128, 128], bf16, tag="w2rep", name="w2rep")
        nc.vector.tensor_copy(out=w2rep[:], in_=w2rf[:])
        cb2_t = consts.tile([128, 1], f32, tag="cb2", name="cb2")
        nc.scalar.dma_start(cb2_t[:], cb2_d.ap())
        psw_ctx.close()

        zps = ctx.enter_context(tc.tile_pool(name="zps", bufs=2, space="PSUM"))
        pst = ctx.enter_context(tc.tile_pool(name="pst", bufs=3, space="PSUM"))

        def emit_floor(dst, srcap, nm):
            """dst = floor(srcap), srcap in [0, 32); robust to convert rounding."""
            ti = sm.tile([128, Q], i16, tag="flt_i", name=f"fi_{nm}")
            tf = sm.tile([128, Q], f32, tag="flt_f", name=f"ff_{nm}")
            nc.vector.tensor_copy(out=ti[:], in_=srcap)
            nc.vector.tensor_copy(out=dst, in_=ti[:])
            nc.vector.tensor_tensor(out=tf[:], in0=dst, in1=srcap, op=OP.is_gt)
            nc.vector.tensor_tensor(out=dst, in0=dst, in1=tf[:], op=OP.subtract)

        for i in range(IMGS):
            # ---------------- uv prep: weights + wrapped idx ----------------
            # vert j at (partition j%128, col j//128)
            uvt = sm.tile([128, 2 * Q], f32, tag="uvt", name=f"uvt{i}")
            uv_i = uv_d.ap()[i]
            nc.scalar.dma_start(
                uvt[:],
                AP(uv_i.tensor, uv_i.offset, [[2, 128], [256, Q], [1, 2]]),
            )
            px = sm.tile([128, Q], f32, tag="px", name=f"px{i}")
            py = sm.tile([128, Q], f32, tag="py", name=f"py{i}")
            nc.vector.tensor_scalar(out=px[:], in0=uvt[:, 0 : 2 * Q : 2],
                                    scalar1=15.5, scalar2=15.5, op0=OP.mult, op1=OP.add)
            nc.vector.tensor_scalar(out=py[:], in0=uvt[:, 1 : 2 * Q : 2],
                                    scalar1=15.5, scalar2=15.5, op0=OP.mult, op1=OP.add)
            x0 = sm.tile([128, Q], f32, tag="x0", name=f"x0{i}")
            y0 = sm.tile([128, Q], f32, tag="y0", name=f"y0{i}")
            emit_floor(x0[:], px[:], f"x{i}")
            emit_floor(y0[:], py[:], f"y{i}")
            nc.vector.tensor_scalar(out=x0[:], in0=x0[:], scalar1=30.0, scalar2=0.0,
                                    op0=OP.min, op1=OP.max)
            nc.vector.tensor_scalar(out=y0[:], in0=y0[:], scalar1=30.0, scalar2=0.0,
                                    op0=OP.min, op1=OP.max)
            wxf = sm.tile([128, Q], f32, tag="wxf", name=f"wxf{i}")
            wyf = sm.tile([128, Q], f32, tag="wyf", name=f"wyf{i}")
            nc.vector.tensor_tensor(out=wxf[:], in0=px[:], in1=x0[:], op=OP.subtract)
            nc.vector.tensor_tensor(out=wyf[:], in0=py[:], in1=y0[:], op=OP.subtract)
            wx = irp.tile([128, Q], bf16, tag="wx", name=f"wx{i}")
            wy = irp.tile([128, Q], bf16, tag="wy", name=f"wy{i}")
            nc.vector.tensor_copy(out=wx[:], in_=wxf[:])
            nc.vector.tensor_copy(out=wy[:], in_=wyf[:])
            idxf = sm.tile([128, Q], f32, tag="idxf", name=f"idxf{i}")
            nc.vector.scalar_tensor_tensor(
                out=idxf[:], in0=y0[:], scalar=32.0, in1=x0[:],
                op0=OP.mult, op1=OP.add,
            )
            idxi = irp.tile([128, Q], i32, tag="idxi", name=f"idxi{i}")
            nc.vector.tensor_copy(out=idxi[:], in_=idxf[:])

            # ---------------- z at pixels (PE) ----------------
            ft = featp.tile([128, NCH * PIX], bf16, tag="ft", name=f"ft{i}")
            f_i = feat_d.ap()[i]
            nc.sync.dma_start(
                ft[:],
                AP(f_i.tensor, f_i.offset,
                   [[PIX, 128], [128 * PIX, NCH], [1, PIX]]),
            )
            zp = zps.tile([128, PIX], f32, tag="zp", name=f"zp{i}")
            for ph in range(2):
                sl = slice(512 * ph, 512 * (ph + 1))
                for c in range(NCH):
                    nc.tensor.matmul(
                        zp[:, sl],
                        lhsT=Wt[c][:],
                        rhs=ft[:, PIX * c + 512 * ph : PIX * c + 512 * (ph + 1)],
                        start=(c == 0),
                        stop=False,
                        skip_group_check=True,
                    )
                nc.tensor.matmul(
                    zp[:, sl], lhsT=bbias[:], rhs=ones_t[:, sl],
                    start=False, stop=True, skip_group_check=True,
                )

            # escape + pre-differenced quantities (dims-major, bf16)
            zq = zqp.tile([128, PPAD], bf16, tag="zq", name=f"zq{i}")
            dzx = zqp.tile([128, PPAD], bf16, tag="zq", name=f"dzx{i}")
            dzy = zqp.tile([128, PPAD], bf16, tag="zq", name=f"dzy{i}")
            dzxy = zqp.tile([128, PPAD], bf16, tag="zq", name=f"dzxy{i}")
            nc.scalar.copy(zq[:, 0:PIX], zp[:])
            nc.vector.memset(zq[:, PIX:PPAD], 0.0)
            nc.vector.tensor_tensor(out=dzx[:, 0:1056], in0=zq[:, 1:1057],
                                    in1=zq[:, 0:1056], op=OP.subtract)
            nc.vector.memset(dzx[:, 1056:PPAD], 0.0)
            nc.vector.tensor_tensor(out=dzy[:, 0:1056], in0=zq[:, 32:PPAD],
                                    in1=zq[:, 0:1056], op=OP.subtract)
            nc.vector.memset(dzy[:, 1056:PPAD], 0.0)
            nc.vector.tensor_tensor(out=dzxy[:, 0:1055], in0=dzy[:, 1:1056],
                                    in1=dzy[:, 0:1055], op=OP.subtract)
            nc.vector.memset(dzxy[:, 1055:PPAD], 0.0)

            # ---------------- tokens to DRAM (PE transpose per 128-pix block) ----
            stg = featp.tile([128, 8 * TOK], bf16, tag="stg", name=f"stg{i}")
            for b in range(8):
                pt = pst.tile([128, TOK], bf16, tag="pt", name=f"pt{i}_{b}")
                for qi, zt in enumerate((zq, dzx, dzy, dzxy)):
                    nc.tensor.transpose(
                        pt[:, 128 * qi : 128 * (qi + 1)],
                        zt[:, 128 * b : 128 * (b + 1)],
                        ident[:],
                    )
                nc.scalar.copy(stg[:, TOK * b : TOK * (b + 1)], pt[:])
            zt_i = ztok_d[i].ap()
            nc.sync.dma_start(
                AP(zt_i.tensor, zt_i.offset,
                   [[TOK, 128], [128 * TOK, 8], [1, TOK]]),
                stg[:].rearrange("p (b t) -> p b t", t=TOK),
            )

            # ---------------- gather + blend + dot per vert chunk ----------------
            logit = lg.tile([128, Q], f32, tag="logit", name=f"lg{i}")
            for ck in range(VCH):
                gt = gpool.tile([128, VROW * TOK], bf16, tag="g", name=f"g{i}_{ck}")
                g3 = gt[:].rearrange("p (r t) -> p r t", t=TOK)
                for r in range(VROW):
                    nc.gpsimd.indirect_dma_start(
                        out=g3[:, r, :],
                        out_offset=None,
                        in_=ztok_d[i].ap(),
                        in_offset=IndirectOffsetOnAxis(
                            ap=idxi[:, VROW * ck + r : VROW * ck + r + 1], axis=0
                        ),
                    )

                def wap(wtile, ck=ck):
                    a = wtile[:]
                    return AP(
                        a.tensor,
                        a.offset + VROW * ck * a.ap[-1][0],
                        [[a.ap[0][0], 128], [a.ap[-1][0], VROW], [0, 128]],
                    )

                t1 = tpool.tile([128, VROW * 128], bf16, tag="t1", name=f"t1_{i}_{ck}")
                t13 = t1[:].rearrange("p (r d) -> p r d", d=128)
                acc = tpool.tile([128, VROW * 128], bf16, tag="acc", name=f"ac{i}_{ck}")
                acc3 = acc[:].rearrange("p (r d) -> p r d", d=128)
                # t1 = wx*dzx ; acc = z00 + t1
                nc.vector.tensor_tensor(out=t13, in0=g3[:, :, 128:256], in1=wap(wx), op=OP.mult)
                nc.vector.tensor_tensor(out=acc3, in0=g3[:, :, 0:128], in1=t13, op=OP.add)
                # t1 = wx*dzxy ; t1 += dzy ; t1 *= wy ; acc += t1
                nc.vector.tensor_tensor(out=t13, in0=g3[:, :, 384:512], in1=wap(wx), op=OP.mult)
                nc.vector.tensor_tensor(out=t13, in0=g3[:, :, 256:384], in1=t13, op=OP.add)
                nc.vector.tensor_tensor(out=t13, in0=t13, in1=wap(wy), op=OP.mult)
                nc.vector.tensor_tensor(out=acc3, in0=acc3, in1=t13, op=OP.add)
                # h = relu(acc) * w2   (fused), then reduce over dims
                w2ap = AP(
                    w2rep[:].tensor, w2rep[:].offset,
                    [[w2rep[:].ap[0][0], 128], [0, VROW], [1, 128]],
                )
                nc.vector.scalar_tensor_tensor(
                    out=acc3, in0=acc3, scalar=0.0, in1=w2ap,
                    op0=OP.max, op1=OP.mult,
                )
                nc.vector.tensor_reduce(
                    out=logit[:, VROW * ck : VROW * (ck + 1)].rearrange(
                        "p (r one) -> p r one", one=1
                    ),
                    in_=acc3,
                    axis=mybir.AxisListType.X,
                    op=OP.add,
                )
            ostg = lg.tile([128, Q], f32, tag="ostg", name=f"os{i}")
            nc.scalar.activation(ostg[:], logit[:], ACT.Sigmoid, bias=cb2_t[:])
            o_i = out_d.ap()[i]
            oap = AP(o_i.tensor, o_i.offset, [[1, 128], [128, Q]])
            nc.scalar.dma_start(oap, ostg[:])

    nc.compile()
    _CACHE["nc"] = nc
    return nc


def _host_prep(inputs):
    feat = np.asarray(inputs["feat_map"], dtype=np.float32)
    uv = np.asarray(inputs["verts_uv"], dtype=np.float32)
    rw = np.asarray(inputs["reduce_w"], dtype=np.float32)
    rb = np.asarray(inputs["reduce_b"], dtype=np.float32)
    w1 = np.asarray(inputs["cls_w1"], dtype=np.float32)
    b1 = np.asarray(inputs["cls_b1"], dtype=np.float32)
    w2 = np.asarray(inputs["cls_w2"], dtype=np.float32)
    b2 = np.asarray(inputs["cls_b2"], dtype=np.float32)

    rwt = np.ascontiguousarray(rw.T)                      # (256, 1280)
    uvp = np.zeros((B, NV, 2), dtype=np.float32)
    uvp[:, :N, :] = uv
    featr = feat.reshape(B, C, PIX).astype(ml_dtypes.bfloat16)

    shared = {
        "rwt": rwt,
        "cw1": np.ascontiguousarray(w1),
        "rb": rb,
        "cb1": b1,
        "w2r": np.ascontiguousarray(np.tile(w2[None, :], (128, 1))),
        "cb2": np.full((128, 1), b2[0], dtype=np.float32),
        "ident": np.eye(128, dtype=ml_dtypes.bfloat16),
    }
    in_maps = []
    for core in range(NCORES):
        sl = slice(core * IMGS, (core + 1) * IMGS)
        m = dict(shared)
        m["feat"] = np.ascontiguousarray(featr[sl])
        m["uv"] = np.ascontiguousarray(uvp[sl])
        in_maps.append(m)
    return in_maps


def kernel(**inputs):
    from concourse.bass_utils import run_bass_kernel_spmd

    nc = _build()
    in_maps = _host_prep(inputs)
    res = run_bass_kernel_spmd(nc, in_maps, list(range(NCORES)))
    out = np.empty((B, N), dtype=np.float32)
    for core in range(NCORES):
        dev = res.results[core]["out"]          # (IMGS, NV), vert j at col j
        out[core * IMGS : (core + 1) * IMGS] = dev[:, :N]
    return out



# revision 5
# speedup vs baseline: 1.3680x; 1.3680x over previous
"""nn_ContactHead Trainium2 kernel (8-core data parallel).

out = sigmoid(w2 . relu((grid_sample(feat, uv) @ reduce_w + reduce_b) @ cls_w1 + cls_b1) + cls_b2)

Everything left of the relu is linear and bilinear sampling is linear in the
features, so the channel reductions commute with the sampling:
  W  = reduce_w @ cls_w1            (1280 x 128)   [device, PE]
  bb = reduce_b @ cls_w1 + cls_b1   (128)          [device, PE via ones-row]
  z[d, pix] = feat[:, pix].W[:, d] + bb[d]    at the 1024 pixels (PE, bf16)
Bilinear via pre-differenced pixel quantities (one gather row per vert):
  dzx = z(x+1)-z ; dzy = z(y+1)-z ; dzxy = dzy(x+1)-dzy
  v(wx,wy) = z00 + wx*dzx + wy*(dzy + wx*dzxy)
Tokens [z00|dzx|dzy|dzxy] (1KB bf16 rows, pixel-major) are written to DRAM
(PE transpose), then fetched with the ant-custom dma_gather: ONE instruction
per half-image (3456 int16 indices, wrapped-16 layout, computed on host along
with the bilinear weights; SWDGE cost ~1us + 0.34ns/desc).
Blend on DVE with *pair-duplicated* weight tiles: each weight stored twice
adjacently so the broadcast AP ends in [1,2] (step-1, 4B-aligned) and
tensor_tensor runs in 2x_1P packed mode (HW-validated). relu+w2 fused via
scalar_tensor_tensor, dot via tensor_reduce, sigmoid on ACT.

Vert layout: vert j lives at (partition j%128, column j//128).
Outputs are stored partition-major and unshuffled on host.
"""

import ml_dtypes
import numpy as np

B, C, H, W, N = 32, 1280, 32, 32, 6890
NCORES = 8
IMGS = B // NCORES          # 4 images per core
PIX = H * W                 # 1024
PPAD = 1088                 # padded pixel slots in the dims-major z tiles
NCH = C // 128              # 10 channel chunks
MID = 128
NV = 6912                   # padded verts (= 54*128)
Q = NV // 128               # 54
GH = 2                      # dma_gather calls per image
GNV = NV // GH              # 3456 verts per gather
GQ = GNV // 128             # 27 cols per gather tile
VCH = 6                     # blend chunks per image
VROW = 9                    # cols per blend chunk
TOK = 512                   # token row: 4 quantities x 128 dims (bf16)

_CACHE = {}


def _build():
    if "nc" in _CACHE:
        return _CACHE["nc"]

    from contextlib import ExitStack

    import concourse.bass as bass
    import concourse.tile as tile
    from concourse import bacc, mybir
    from concourse.ap import AP

    f32 = mybir.dt.float32
    bf16 = mybir.dt.bfloat16
    i16 = mybir.dt.int16
    OP = mybir.AluOpType
    ACT = mybir.ActivationFunctionType

    nc = bacc.Bacc("TRN2", target_bir_lowering=False, debug=False)

    feat_d = nc.dram_tensor("feat", [IMGS, C, PIX], bf16, kind="ExternalInput")
    wxd_d = nc.dram_tensor("wxd", [IMGS, 128, 2 * Q], bf16, kind="ExternalInput")
    wyd_d = nc.dram_tensor("wyd", [IMGS, 128, 2 * Q], bf16, kind="ExternalInput")
    idx_d = nc.dram_tensor("idx", [IMGS, 128, NV // 16], i16, kind="ExternalInput")
    rwt_d = nc.dram_tensor("rwt", [256, C], f32, kind="ExternalInput")
    cw1_d = nc.dram_tensor("cw1", [256, MID], f32, kind="ExternalInput")
    rb_d = nc.dram_tensor("rb", [256], f32, kind="ExternalInput")
    cb1_d = nc.dram_tensor("cb1", [MID], f32, kind="ExternalInput")
    w2r_d = nc.dram_tensor("w2r", [128, 128], f32, kind="ExternalInput")
    cb2_d = nc.dram_tensor("cb2", [128, 1], f32, kind="ExternalInput")
    id_d = nc.dram_tensor("ident", [128, 128], bf16, kind="ExternalInput")
    ztok_d = [
        nc.dram_tensor(f"ztok{i}", [PIX, TOK], bf16) for i in range(IMGS)
    ]
    out_d = nc.dram_tensor("out", [IMGS, 128, Q], f32, kind="ExternalOutput")

    with tile.TileContext(nc) as tc, ExitStack() as ctx:
        consts = ctx.enter_context(tc.tile_pool(name="consts", bufs=1))
        prep = ctx.enter_context(tc.tile_pool(name="prep", bufs=1))
        featp = ctx.enter_context(tc.tile_pool(name="featp", bufs=2))
        zqp = ctx.enter_context(tc.tile_pool(name="zqp", bufs=8))
        gpool = ctx.enter_context(tc.tile_pool(name="gpool", bufs=2))
        tpool = ctx.enter_context(tc.tile_pool(name="tpool", bufs=4))
        irp = ctx.enter_context(tc.tile_pool(name="irp", bufs=4))
        lg = ctx.enter_context(tc.tile_pool(name="lg", bufs=2))

        # ---------------- phase 0: combined weights (PE) ----------------
        psw_ctx = ExitStack()
        psw = psw_ctx.enter_context(tc.tile_pool(name="psw", bufs=2, space="PSUM"))
        rwt_t, cw1_t = [], []
        for k in range(2):
            rt = prep.tile([128, C], f32, tag=f"rwt{k}", name=f"rwt{k}")
            nc.sync.dma_start(rt[:], rwt_d.ap()[128 * k : 128 * (k + 1), :])
            rwt_t.append(rt)
            ct = prep.tile([128, MID], f32, tag=f"cw1{k}", name=f"cw1{k}")
            nc.sync.dma_start(ct[:], cw1_d.ap()[128 * k : 128 * (k + 1), :])
            cw1_t.append(ct)

        Wt = []
        for c in range(NCH):
            pw = psw.tile([128, 128], f32, tag="pw", name=f"pw{c}")
            for k in range(2):
                nc.tensor.matmul(
                    pw[:],
                    lhsT=rwt_t[k][:, 128 * c : 128 * (c + 1)],
                    rhs=cw1_t[k][:],
                    start=(k == 0),
                    stop=(k == 1),
                )
            wt = consts.tile([128, 128], bf16, tag=f"W{c}", name=f"W{c}")
            nc.scalar.copy(wt[:], pw[:])
            Wt.append(wt)

        rb_t = prep.tile([128, 2], f32, tag="rb", name="rb")
        nc.scalar.dma_start(rb_t[:], rb_d.ap().rearrange("(k p) -> p k", p=128))
        cb1_t = prep.tile([1, MID], f32, tag="cb1", name="cb1")
        nc.scalar.dma_start(cb1_t[:], cb1_d.ap().rearrange("(one d) -> one d", one=1))
        pb = psw.tile([1, 128], f32, tag="pb", name="pb")
        for k in range(2):
            nc.tensor.matmul(
                pb[:], lhsT=rb_t[:, k : k + 1], rhs=cw1_t[k][:],
                start=(k == 0), stop=(k == 1),
            )
        brow = prep.tile([1, 128], f32, tag="brow", name="brow")
        nc.vector.tensor_tensor(out=brow[:], in0=pb[:], in1=cb1_t[:], op=OP.add)
        bbias = consts.tile([1, 128], bf16, tag="bbias", name="bbias")
        nc.scalar.copy(bbias[:], brow[:])

        ones_t = consts.tile([1, PIX], bf16, tag="ones", name="ones")
        nc.vector.memset(ones_t[:], 1.0)
        ident = consts.tile([128, 128], bf16, tag="ident", name="ident")
        nc.scalar.dma_start(ident[:], id_d.ap())
        w2rf = prep.tile([128, 128], f32, tag="w2rf", name="w2rf")
        nc.scalar.dma_start(w2rf[:], w2r_d.ap())
        w2rep = consts.tile([128, 128], bf16, tag="w2rep", name="w2rep")
        nc.vector.tensor_copy(out=w2rep[:], in_=w2rf[:])
        cb2_t = consts.tile([128, 1], f32, tag="cb2", name="cb2")
        nc.scalar.dma_start(cb2_t[:], cb2_d.ap())
        psw_ctx.close()

        zps = ctx.enter_context(tc.tile_pool(name="zps", bufs=2, space="PSUM"))
        pst = ctx.enter_context(tc.tile_pool(name="pst", bufs=3, space="PSUM"))

        for i in range(IMGS):
            # host-computed bilinear weights (dup-pairs) + wrapped-16 indices
            wxd = irp.tile([128, 2 * Q], bf16, tag="wxd", name=f"wxd{i}")
            wyd = irp.tile([128, 2 * Q], bf16, tag="wyd", name=f"wyd{i}")
            idxt = irp.tile([128, NV // 16], i16, tag="idx", name=f"idx{i}")
            nc.scalar.dma_start(wxd[:], wxd_d.ap()[i])
            nc.scalar.dma_start(wyd[:], wyd_d.ap()[i])
            nc.scalar.dma_start(idxt[:], idx_d.ap()[i])

            # ---------------- z at pixels (PE) ----------------
            ft = featp.tile([128, NCH * PIX], bf16, tag="ft", name=f"ft{i}")
            f_i = feat_d.ap()[i]
            nc.sync.dma_start(
                ft[:],
                AP(f_i.tensor, f_i.offset,
                   [[PIX, 128], [128 * PIX, NCH], [1, PIX]]),
            )
            zp = zps.tile([128, PIX], f32, tag="zp", name=f"zp{i}")
            for ph in range(2):
                sl = slice(512 * ph, 512 * (ph + 1))
                for c in range(NCH):
                    nc.tensor.matmul(
                        zp[:, sl],
                        lhsT=Wt[c][:],
                        rhs=ft[:, PIX * c + 512 * ph : PIX * c + 512 * (ph + 1)],
                        start=(c == 0),
                        stop=False,
                        skip_group_check=True,
                    )
                nc.tensor.matmul(
                    zp[:, sl], lhsT=bbias[:], rhs=ones_t[:, sl],
                    start=False, stop=True, skip_group_check=True,
                )

            # escape + pre-differenced quantities (dims-major, bf16)
            zq = zqp.tile([128, PPAD], bf16, tag="zq", name=f"zq{i}")
            dzx = zqp.tile([128, PPAD], bf16, tag="zq", name=f"dzx{i}")
            dzy = zqp.tile([128, PPAD], bf16, tag="zq", name=f"dzy{i}")
            dzxy = zqp.tile([128, PPAD], bf16, tag="zq", name=f"dzxy{i}")
            nc.scalar.copy(zq[:, 0:PIX], zp[:])
            nc.vector.memset(zq[:, PIX:PPAD], 0.0)
            nc.vector.tensor_tensor(out=dzx[:, 0:1056], in0=zq[:, 1:1057],
                                    in1=zq[:, 0:1056], op=OP.subtract)
            nc.vector.memset(dzx[:, 1056:PPAD], 0.0)
            nc.vector.tensor_tensor(out=dzy[:, 0:1056], in0=zq[:, 32:PPAD],
                                    in1=zq[:, 0:1056], op=OP.subtract)
            nc.vector.memset(dzy[:, 1056:PPAD], 0.0)
            nc.vector.tensor_tensor(out=dzxy[:, 0:1055], in0=dzy[:, 1:1056],
                                    in1=dzy[:, 0:1055], op=OP.subtract)
            nc.vector.memset(dzxy[:, 1055:PPAD], 0.0)

            # ---------------- tokens to DRAM (PE transpose per 128-pix block) ----
            stg = featp.tile([128, 8 * TOK], bf16, tag="stg", name=f"stg{i}")
            for b in range(8):
                pt = pst.tile([128, TOK], bf16, tag="pt", name=f"pt{i}_{b}")
                for qi, zt in enumerate((zq, dzx, dzy, dzxy)):
                    nc.tensor.transpose(
                        pt[:, 128 * qi : 128 * (qi + 1)],
                        zt[:, 128 * b : 128 * (b + 1)],
                        ident[:],
                    )
                nc.scalar.copy(stg[:, TOK * b : TOK * (b + 1)], pt[:])
            zt_i = ztok_d[i].ap()
            nc.sync.dma_start(
                AP(zt_i.tensor, zt_i.offset,
                   [[TOK, 128], [128 * TOK, 8], [1, TOK]]),
                stg[:].rearrange("p (b t) -> p b t", t=TOK),
            )

            # ---------------- gather (2 per image) + blend + dot ----------------
            logit = lg.tile([128, Q], f32, tag="logit", name=f"lg{i}")
            for h in range(GH):
                gt = gpool.tile([128, GQ * TOK], bf16, tag="g", name=f"g{i}_{h}")
                gt3 = gt[:].rearrange("p (c t) -> p c t", t=TOK)
                nc.gpsimd.dma_gather(
                    out_ap=gt3,
                    in_ap=ztok_d[i].ap(),
                    idxs_ap=idxt[:, h * (GNV // 16) : (h + 1) * (GNV // 16)],
                    num_idxs=GNV,
                    num_idxs_reg=GNV,
                    elem_size=TOK,
                    single_packet=False,
                )
                for j in range(VCH // GH):
                    ck = h * (VCH // GH) + j          # global chunk id (9 cols)
                    g3 = gt3[:, VROW * j : VROW * (j + 1), :]

                    def wap(wtile, ck=ck):
                        # dup-pair weight broadcast: [[p,128],[2,VROW],[0,64],[1,2]]
                        a = wtile[:]
                        return AP(
                            a.tensor,
                            a.offset + 2 * VROW * ck * a.ap[-1][0],
                            [[a.ap[0][0], 128], [2 * a.ap[-1][0], VROW],
                             [0, 64], [a.ap[-1][0], 2]],
                        )

                    def p2(apv):
                        # view [128, VROW, 128] as [128, VROW, 64, 2] (same memory)
                        return apv.rearrange("p r (d2 k) -> p r d2 k", k=2)

                    t1 = tpool.tile([128, VROW * 128], bf16, tag="t1", name=f"t1_{i}_{ck}")
                    t13 = t1[:].rearrange("p (r d) -> p r d", d=128)
                    acc = tpool.tile([128, VROW * 128], bf16, tag="acc", name=f"ac{i}_{ck}")
                    acc3 = acc[:].rearrange("p (r d) -> p r d", d=128)
                    # t1 = wx*dzx ; acc = z00 + t1
                    nc.vector.tensor_tensor(out=p2(t13), in0=p2(g3[:, :, 128:256]), in1=wap(wxd), op=OP.mult)
                    nc.vector.tensor_tensor(out=acc3, in0=g3[:, :, 0:128], in1=t13, op=OP.add)
                    # t1 = wx*dzxy ; t1 += dzy ; t1 *= wy ; acc += t1
                    nc.vector.tensor_tensor(out=p2(t13), in0=p2(g3[:, :, 384:512]), in1=wap(wxd), op=OP.mult)
                    nc.vector.tensor_tensor(out=t13, in0=g3[:, :, 256:384], in1=t13, op=OP.add)
                    nc.vector.tensor_tensor(out=p2(t13), in0=p2(t13), in1=wap(wyd), op=OP.mult)
                    nc.vector.tensor_tensor(out=acc3, in0=acc3, in1=t13, op=OP.add)
                    # h = relu(acc) * w2   (fused), then reduce over dims
                    w2ap = AP(
                        w2rep[:].tensor, w2rep[:].offset,
                        [[w2rep[:].ap[0][0], 128], [0, VROW], [1, 128]],
                    )
                    nc.vector.scalar_tensor_tensor(
                        out=acc3, in0=acc3, scalar=0.0, in1=w2ap,
                        op0=OP.max, op1=OP.mult,
                    )
                    nc.vector.tensor_reduce(
                        out=logit[:, VROW * ck : VROW * (ck + 1)].rearrange(
                            "p (r one) -> p r one", one=1
                        ),
                        in_=acc3,
                        axis=mybir.AxisListType.X,
                        op=OP.add,
                    )
            ostg = lg.tile([128, Q], f32, tag="ostg", name=f"os{i}")
            nc.scalar.activation(ostg[:], logit[:], ACT.Sigmoid, bias=cb2_t[:])
            nc.scalar.dma_start(out_d.ap()[i], ostg[:])

    nc.compile()
    _CACHE["nc"] = nc
    return nc


def _host_prep(inputs):
    feat = np.asarray(inputs["feat_map"], dtype=np.float32)
    uv = np.asarray(inputs["verts_uv"], dtype=np.float32)
    rw = np.asarray(inputs["reduce_w"], dtype=np.float32)
    rb = np.asarray(inputs["reduce_b"], dtype=np.float32)
    w1 = np.asarray(inputs["cls_w1"], dtype=np.float32)
    b1 = np.asarray(inputs["cls_b1"], dtype=np.float32)
    w2 = np.asarray(inputs["cls_w2"], dtype=np.float32)
    b2 = np.asarray(inputs["cls_b2"], dtype=np.float32)

    rwt = np.ascontiguousarray(rw.T)                      # (256, 1280)
    uvp = np.zeros((B, NV, 2), dtype=np.float32)
    uvp[:, :N, :] = uv

    # bilinear weights + base-pixel index (same clamped-floor math as device v1)
    px = (uvp[:, :, 0] + 1.0) * np.float32(15.5)
    py = (uvp[:, :, 1] + 1.0) * np.float32(15.5)
    x0 = np.clip(np.floor(px), 0.0, 30.0)
    y0 = np.clip(np.floor(py), 0.0, 30.0)
    wx = (px - x0).astype(ml_dtypes.bfloat16)             # (B, NV)
    wy = (py - y0).astype(ml_dtypes.bfloat16)
    idx = (y0 * 32 + x0).astype(np.int16)                 # (B, NV) in [0, 990]

    # vert j at (partition j%128, col j//128); dup-pair along cols
    def dup_pair(w):
        wl = w.reshape(B, Q, 128).transpose(0, 2, 1)      # (B, 128, Q)
        return np.ascontiguousarray(np.repeat(wl, 2, axis=2))   # (B, 128, 2Q)

    wxd = dup_pair(wx)
    wyd = dup_pair(wy)

    # wrapped-16 idx layout per gather half, replicated across the 8 core groups
    idx_w = np.empty((B, 128, NV // 16), dtype=np.int16)
    hw = GNV // 16
    for h in range(GH):
        sub = idx[:, h * GNV : (h + 1) * GNV]             # (B, GNV)
        wrap = sub.reshape(B, hw, 16).transpose(0, 2, 1)  # (B, 16, hw)
        idx_w[:, :, h * hw : (h + 1) * hw] = np.tile(wrap, (1, 8, 1))

    featr = feat.reshape(B, C, PIX).astype(ml_dtypes.bfloat16)

    shared = {
        "rwt": rwt,
        "cw1": np.ascontiguousarray(w1),
        "rb": rb,
        "cb1": b1,
        "w2r": np.ascontiguousarray(np.tile(w2[None, :], (128, 1))),
        "cb2": np.full((128, 1), b2[0], dtype=np.float32),
        "ident": np.eye(128, dtype=ml_dtypes.bfloat16),
    }
    in_maps = []
    for core in range(NCORES):
        sl = slice(core * IMGS, (core + 1) * IMGS)
        m = dict(shared)
        m["feat"] = np.ascontiguousarray(featr[sl])
        m["wxd"] = np.ascontiguousarray(wxd[sl])
        m["wyd"] = np.ascontiguousarray(wyd[sl])
        m["idx"] = np.ascontiguousarray(idx_w[sl])
        in_maps.append(m)
    return in_maps


def kernel(**inputs):
    from concourse.bass_utils import run_bass_kernel_spmd

    nc = _build()
    in_maps = _host_prep(inputs)
    res = run_bass_kernel_spmd(nc, in_maps, list(range(NCORES)))
    out = np.empty((B, N), dtype=np.float32)
    for core in range(NCORES):
        dev = res.results[core]["out"]          # (IMGS, 128, Q), vert j at (j%128, j//128)
        full = dev.transpose(0, 2, 1).reshape(IMGS, NV)
        out[core * IMGS : (core + 1) * IMGS] = full[:, :N]
    return out


# revision 8
# speedup vs baseline: 1.6059x; 1.1739x over previous
"""nn_ContactHead Trainium2 kernel (8-core data parallel).

out = sigmoid(w2 . relu((grid_sample(feat, uv) @ reduce_w + reduce_b) @ cls_w1 + cls_b1) + cls_b2)

Everything left of the relu is linear and bilinear sampling is linear in the
features, so the channel reductions commute with the sampling; additionally the
signed w2 is folded into the combined weights with dims permuted so that
w2>=0 dims come first (P of them):
  W  = reduce_w @ cls_w1[:, perm] * w2[perm]     (1280 x 128)  [device, PE]
  bb = (reduce_b @ cls_w1 + cls_b1)[perm]*w2[perm]             [device, PE]
  logit = sum_{d<P} max(v'_d, 0) + sum_{d>=P} min(v'_d, 0) + b2
(relu(v)*w2 == max(v*w2,0) for w2>=0 and min(v*w2,0) for w2<0.)

z[d,pix] at the 1024 pixels via PE (bf16), then pre-differenced quantities
  dzx = z(x+1)-z ; dzy = z(y+1)-z ; dzxy = dzy(x+1)-dzy
  v(wx,wy) = z00 + wx*dzx + wy*(dzy + wx*dzxy)
Token rows [z00|dzy|dzx|dzxy] (1KB bf16, pixel-major) go to DRAM (PE
transpose) and come back with dma_gather (1024 int16 idx / call,
single_packet, wrapped-16 idx layout computed on host with the bilinear
weights). Blend on DVE in 2x_1P packed mode using pair-duplicated weights
(broadcast AP ends in [1,2]); the two wx multiplies (and the two follow-up
adds) are fused into double-width ops thanks to the token order. Segment
max/min on tensor_scalar (4x mode), two 2x fold-adds halve the reduce input,
tensor_reduce (1x) finishes the dot. Sigmoid+bias on ACT.

Vert layout: vert j lives at (partition j%128, column j//128).
Outputs are stored partition-major and unshuffled on host.
"""

import ml_dtypes
import numpy as np

B, C, H, W, N = 32, 1280, 32, 32, 6890
NCORES = 8
IMGS = B // NCORES          # 4 images per core
PIX = H * W                 # 1024
PPAD = 1088                 # padded pixel slots in the dims-major z tiles
NCH = C // 128              # 10 channel chunks
MID = 128
NV = 6912                   # padded verts (= 54*128)
Q = NV // 128               # 54
TOK = 512                   # token row: 4 quantities x 128 dims (bf16)
GCOLS = [8, 8, 8, 8, 8, 8, 6]          # cols per dma_gather (<=8 -> <=1024 idx)
CHUNKS = [(0, 16), (16, 16), (32, 16), (48, 6)]   # blend (col0, ncols)

_CACHE = {}


def _build(P):
    key = ("nc", P)
    if key in _CACHE:
        return _CACHE[key]

    from contextlib import ExitStack

    import concourse.bass as bass
    import concourse.tile as tile
    from concourse import bacc, mybir
    from concourse.ap import AP

    f32 = mybir.dt.float32
    bf16 = mybir.dt.bfloat16
    i16 = mybir.dt.int16
    OP = mybir.AluOpType
    ACT = mybir.ActivationFunctionType

    nc = bacc.Bacc("TRN2", target_bir_lowering=False, debug=False)

    feat_d = nc.dram_tensor("feat", [IMGS, C, PIX], bf16, kind="ExternalInput")
    wxd_d = nc.dram_tensor("wxd", [IMGS, 128, 2 * Q], bf16, kind="ExternalInput")
    wyd_d = nc.dram_tensor("wyd", [IMGS, 128, 2 * Q], bf16, kind="ExternalInput")
    idx_d = nc.dram_tensor("idx", [IMGS, 128, NV // 16], i16, kind="ExternalInput")
    rwt_d = nc.dram_tensor("rwt", [256, C], f32, kind="ExternalInput")
    cw1_d = nc.dram_tensor("cw1", [256, MID], f32, kind="ExternalInput")
    rb_d = nc.dram_tensor("rb", [256], f32, kind="ExternalInput")
    cb1_d = nc.dram_tensor("cb1", [MID], f32, kind="ExternalInput")
    cb2_d = nc.dram_tensor("cb2", [128, 1], f32, kind="ExternalInput")
    id_d = nc.dram_tensor("ident", [128, 128], bf16, kind="ExternalInput")
    ztok_d = [
        nc.dram_tensor(f"ztok{i}", [PIX, TOK], bf16) for i in range(IMGS)
    ]
    out_d = nc.dram_tensor("out", [IMGS, 128, Q], f32, kind="ExternalOutput")

    with tile.TileContext(nc) as tc, ExitStack() as ctx:
        consts = ctx.enter_context(tc.tile_pool(name="consts", bufs=1))
        prep = ctx.enter_context(tc.tile_pool(name="prep", bufs=1))
        featp = ctx.enter_context(tc.tile_pool(name="featp", bufs=2))
        zqp = ctx.enter_context(tc.tile_pool(name="zqp", bufs=8))
        gpool = ctx.enter_context(tc.tile_pool(name="gpool", bufs=2))
        tpool = ctx.enter_context(tc.tile_pool(name="tpool", bufs=2))
        irp = ctx.enter_context(tc.tile_pool(name="irp", bufs=4))
        lg = ctx.enter_context(tc.tile_pool(name="lg", bufs=2))

        # ---------------- phase 0: combined weights (PE) ----------------
        psw_ctx = ExitStack()
        psw = psw_ctx.enter_context(tc.tile_pool(name="psw", bufs=2, space="PSUM"))
        rwt_t, cw1_t = [], []
        for k in range(2):
            rt = prep.tile([128, C], f32, tag=f"rwt{k}", name=f"rwt{k}")
            nc.sync.dma_start(rt[:], rwt_d.ap()[128 * k : 128 * (k + 1), :])
            rwt_t.append(rt)
            ct = prep.tile([128, MID], f32, tag=f"cw1{k}", name=f"cw1{k}")
            nc.sync.dma_start(ct[:], cw1_d.ap()[128 * k : 128 * (k + 1), :])
            cw1_t.append(ct)

        Wt = []
        for c in range(NCH):
            pw = psw.tile([128, 128], f32, tag="pw", name=f"pw{c}")
            for k in range(2):
                nc.tensor.matmul(
                    pw[:],
                    lhsT=rwt_t[k][:, 128 * c : 128 * (c + 1)],
                    rhs=cw1_t[k][:],
                    start=(k == 0),
                    stop=(k == 1),
                )
            wt = consts.tile([128, 128], bf16, tag=f"W{c}", name=f"W{c}")
            nc.scalar.copy(wt[:], pw[:])
            Wt.append(wt)

        rb_t = prep.tile([128, 2], f32, tag="rb", name="rb")
        nc.scalar.dma_start(rb_t[:], rb_d.ap().rearrange("(k p) -> p k", p=128))
        cb1_t = prep.tile([1, MID], f32, tag="cb1", name="cb1")
        nc.scalar.dma_start(cb1_t[:], cb1_d.ap().rearrange("(one d) -> one d", one=1))
        pb = psw.tile([1, 128], f32, tag="pb", name="pb")
        for k in range(2):
            nc.tensor.matmul(
                pb[:], lhsT=rb_t[:, k : k + 1], rhs=cw1_t[k][:],
                start=(k == 0), stop=(k == 1),
            )
        brow = prep.tile([1, 128], f32, tag="brow", name="brow")
        nc.vector.tensor_tensor(out=brow[:], in0=pb[:], in1=cb1_t[:], op=OP.add)
        bbias = consts.tile([1, 128], bf16, tag="bbias", name="bbias")
        nc.scalar.copy(bbias[:], brow[:])

        ones_t = consts.tile([1, PIX], bf16, tag="ones", name="ones")
        nc.vector.memset(ones_t[:], 1.0)
        ident = consts.tile([128, 128], bf16, tag="ident", name="ident")
        nc.scalar.dma_start(ident[:], id_d.ap())
        cb2_t = consts.tile([128, 1], f32, tag="cb2", name="cb2")
        nc.scalar.dma_start(cb2_t[:], cb2_d.ap())
        psw_ctx.close()

        zps = ctx.enter_context(tc.tile_pool(name="zps", bufs=2, space="PSUM"))
        pst = ctx.enter_context(tc.tile_pool(name="pst", bufs=3, space="PSUM"))

        for i in range(IMGS):
            # host-computed bilinear weights (dup-pairs) + wrapped-16 indices
            wxd = irp.tile([128, 2 * Q], bf16, tag="wxd", name=f"wxd{i}")
            wyd = irp.tile([128, 2 * Q], bf16, tag="wyd", name=f"wyd{i}")
            idxt = irp.tile([128, NV // 16], i16, tag="idx", name=f"idx{i}")
            nc.scalar.dma_start(wxd[:], wxd_d.ap()[i])
            nc.scalar.dma_start(wyd[:], wyd_d.ap()[i])
            nc.scalar.dma_start(idxt[:], idx_d.ap()[i])

            # ---------------- z at pixels (PE) ----------------
            ft = featp.tile([128, NCH * PIX], bf16, tag="ft", name=f"ft{i}")
            f_i = feat_d.ap()[i]
            nc.sync.dma_start(
                ft[:],
                AP(f_i.tensor, f_i.offset,
                   [[PIX, 128], [128 * PIX, NCH], [1, PIX]]),
            )
            zp = zps.tile([128, PIX], f32, tag="zp", name=f"zp{i}")
            for ph in range(2):
                sl = slice(512 * ph, 512 * (ph + 1))
                for c in range(NCH):
                    nc.tensor.matmul(
                        zp[:, sl],
                        lhsT=Wt[c][:],
                        rhs=ft[:, PIX * c + 512 * ph : PIX * c + 512 * (ph + 1)],
                        start=(c == 0),
                        stop=False,
                        skip_group_check=True,
                    )
                nc.tensor.matmul(
                    zp[:, sl], lhsT=bbias[:], rhs=ones_t[:, sl],
                    start=False, stop=True, skip_group_check=True,
                )

            # escape + pre-differenced quantities (dims-major, bf16)
            zq = zqp.tile([128, PPAD], bf16, tag="zq", name=f"zq{i}")
            dzx = zqp.tile([128, PPAD], bf16, tag="zq", name=f"dzx{i}")
            dzy = zqp.tile([128, PPAD], bf16, tag="zq", name=f"dzy{i}")
            dzxy = zqp.tile([128, PPAD], bf16, tag="zq", name=f"dzxy{i}")
            nc.scalar.copy(zq[:, 0:PIX], zp[:])
            nc.vector.memset(zq[:, PIX:PPAD], 0.0)
            nc.vector.tensor_tensor(out=dzx[:, 0:1056], in0=zq[:, 1:1057],
                                    in1=zq[:, 0:1056], op=OP.subtract)
            nc.vector.memset(dzx[:, 1056:PPAD], 0.0)
            nc.vector.tensor_tensor(out=dzy[:, 0:1056], in0=zq[:, 32:PPAD],
                                    in1=zq[:, 0:1056], op=OP.subtract)
            nc.vector.memset(dzy[:, 1056:PPAD], 0.0)
            nc.vector.tensor_tensor(out=dzxy[:, 0:1055], in0=dzy[:, 1:1056],
                                    in1=dzy[:, 0:1055], op=OP.subtract)
            nc.vector.memset(dzxy[:, 1055:PPAD], 0.0)

            # --------- tokens [z00|dzy|dzx|dzxy] to DRAM (PE transpose) ---------
            stg = featp.tile([128, 8 * TOK], bf16, tag="stg", name=f"stg{i}")
            for b in range(8):
                pt = pst.tile([128, TOK], bf16, tag="pt", name=f"pt{i}_{b}")
                for qi, zt in enumerate((zq, dzy, dzx, dzxy)):
                    nc.tensor.transpose(
                        pt[:, 128 * qi : 128 * (qi + 1)],
                        zt[:, 128 * b : 128 * (b + 1)],
                        ident[:],
                    )
                nc.scalar.copy(stg[:, TOK * b : TOK * (b + 1)], pt[:])
            zt_i = ztok_d[i].ap()
            nc.sync.dma_start(
                AP(zt_i.tensor, zt_i.offset,
                   [[TOK, 128], [128 * TOK, 8], [1, TOK]]),
                stg[:].rearrange("p (b t) -> p b t", t=TOK),
            )

            # ---------------- gather + blend + dot ----------------
            logit = lg.tile([128, Q], f32, tag="logit", name=f"lg{i}")
            gtiles = {}
            gcol = 0
            for gi, ncols_g in enumerate(GCOLS):
                c0 = gcol
                ct0, szt = CHUNKS[[c for c, (a, s) in enumerate(CHUNKS)
                                   if a <= c0 < a + s][0]]
                if ct0 not in gtiles:
                    gtiles[ct0] = gpool.tile(
                        [128, 16 * TOK], bf16, tag="g", name=f"g{i}_{ct0}"
                    )
                gt3 = gtiles[ct0][:].rearrange("p (c t) -> p c t", t=TOK)
                nc.gpsimd.dma_gather(
                    out_ap=gt3[:, c0 - ct0 : c0 - ct0 + ncols_g, :],
                    in_ap=ztok_d[i].ap(),
                    idxs_ap=idxt[:, c0 * 8 : (c0 + ncols_g) * 8],
                    num_idxs=ncols_g * 128,
                    num_idxs_reg=ncols_g * 128,
                    elem_size=TOK,
                )
                gcol += ncols_g

            for (c0, ncl) in CHUNKS:
                gt3 = gtiles[c0][:].rearrange("p (c t) -> p c t", t=TOK)
                g3 = gt3[:, 0:ncl, :]

                def wap(wtile, npairs, c0=c0, ncl=ncl):
                    # dup-pair weight broadcast: [[p,128],[2,ncl],[0,npairs],[1,2]]
                    a = wtile[:]
                    return AP(
                        a.tensor,
                        a.offset + 2 * c0 * a.ap[-1][0],
                        [[a.ap[0][0], 128], [2 * a.ap[-1][0], ncl],
                         [0, npairs], [a.ap[-1][0], 2]],
                    )

                def pk(apv):
                    # view [..., 2n] as [..., n, 2] (same memory)
                    return apv.rearrange("p r (d2 k) -> p r d2 k", k=2)

                t1w = tpool.tile([128, ncl * 256], bf16, tag=f"t1w{ncl}",
                                 name=f"t1w_{i}_{c0}")
                t1w3 = t1w[:].rearrange("p (r d) -> p r d", d=256)
                a2w = tpool.tile([128, ncl * 256], bf16, tag=f"a2w{ncl}",
                                 name=f"a2w_{i}_{c0}")
                a2w3 = a2w[:].rearrange("p (r d) -> p r d", d=256)
                acc = tpool.tile([128, ncl * 128], bf16, tag=f"acc{ncl}",
                                 name=f"ac_{i}_{c0}")
                acc3 = acc[:].rearrange("p (r d) -> p r d", d=128)
                # [wx*dzx | wx*dzxy]  (one wide 2x op)
                nc.vector.tensor_tensor(out=pk(t1w3), in0=pk(g3[:, :, 256:512]),
                                        in1=wap(wxd, 128), op=OP.mult)
                # [z00+wx*dzx | dzy+wx*dzxy]
                nc.vector.tensor_tensor(out=a2w3, in0=g3[:, :, 0:256],
                                        in1=t1w3, op=OP.add)
                # t3 = wy * (dzy + wx*dzxy)   (reuse t1w first half)
                t3 = t1w3[:, :, 0:128]
                nc.vector.tensor_tensor(out=pk(t3), in0=pk(a2w3[:, :, 128:256]),
                                        in1=wap(wyd, 64), op=OP.mult)
                # v' = (z00+wx*dzx) + t3
                nc.vector.tensor_tensor(out=acc3, in0=a2w3[:, :, 0:128],
                                        in1=t3, op=OP.add)
                # segment rectify: max(.,0) on dims [0,P), min(.,0) on [P,128)
                if P > 0:
                    nc.vector.tensor_scalar(out=acc3[:, :, 0:P], in0=acc3[:, :, 0:P],
                                            scalar1=0.0, scalar2=None, op0=OP.max)
                if P < 128:
                    nc.vector.tensor_scalar(out=acc3[:, :, P:128], in0=acc3[:, :, P:128],
                                            scalar1=0.0, scalar2=None, op0=OP.min)
                # fold 128 -> 32 with 2x adds, then 1x reduce
                u64 = a2w3[:, :, 0:64]
                nc.vector.tensor_tensor(out=u64, in0=acc3[:, :, 0:64],
                                        in1=acc3[:, :, 64:128], op=OP.add)
                u32 = a2w3[:, :, 64:96]
                nc.vector.tensor_tensor(out=u32, in0=u64[:, :, 0:32],
                                        in1=u64[:, :, 32:64], op=OP.add)
                nc.vector.tensor_reduce(
                    out=logit[:, c0 : c0 + ncl].rearrange("p (r one) -> p r one", one=1),
                    in_=u32,
                    axis=mybir.AxisListType.X,
                    op=OP.add,
                )
            ostg = lg.tile([128, Q], f32, tag="ostg", name=f"os{i}")
            nc.scalar.activation(ostg[:], logit[:], ACT.Sigmoid, bias=cb2_t[:])
            nc.scalar.dma_start(out_d.ap()[i], ostg[:])

    nc.compile()
    _CACHE[key] = nc
    return nc


def _host_prep(inputs):
    feat = np.asarray(inputs["feat_map"], dtype=np.float32)
    uv = np.asarray(inputs["verts_uv"], dtype=np.float32)
    rw = np.asarray(inputs["reduce_w"], dtype=np.float32)
    rb = np.asarray(inputs["reduce_b"], dtype=np.float32)
    w1 = np.asarray(inputs["cls_w1"], dtype=np.float32)
    b1 = np.asarray(inputs["cls_b1"], dtype=np.float32)
    w2 = np.asarray(inputs["cls_w2"], dtype=np.float32)
    b2 = np.asarray(inputs["cls_b2"], dtype=np.float32)

    # fold signed w2 into cls_w1/cls_b1, dims permuted sign-sorted (P = #pos)
    perm = np.argsort(w2 < 0, kind="stable")
    P = int((w2 >= 0).sum())
    w1f = np.ascontiguousarray(w1[:, perm] * w2[perm][None, :])
    b1f = np.ascontiguousarray(b1[perm] * w2[perm])

    rwt = np.ascontiguousarray(rw.T)                      # (256, 1280)
    uvp = np.zeros((B, NV, 2), dtype=np.float32)
    uvp[:, :N, :] = uv

    # bilinear weights + base-pixel index (same clamped-floor math as device v1)
    px = (uvp[:, :, 0] + 1.0) * np.float32(15.5)
    py = (uvp[:, :, 1] + 1.0) * np.float32(15.5)
    x0 = np.clip(np.floor(px), 0.0, 30.0)
    y0 = np.clip(np.floor(py), 0.0, 30.0)
    wx = (px - x0).astype(ml_dtypes.bfloat16)             # (B, NV)
    wy = (py - y0).astype(ml_dtypes.bfloat16)
    idx = (y0 * 32 + x0).astype(np.int16)                 # (B, NV) in [0, 990]

    # vert j at (partition j%128, col j//128); dup-pair along cols
    def dup_pair(w):
        wl = w.reshape(B, Q, 128).transpose(0, 2, 1)      # (B, 128, Q)
        return np.ascontiguousarray(np.repeat(wl, 2, axis=2))   # (B, 128, 2Q)

    wxd = dup_pair(wx)
    wyd = dup_pair(wy)

    # wrapped-16 idx layout (whole image), replicated across the 8 core groups;
    # any 128-aligned window is then a contiguous col range
    wrap = idx.reshape(B, NV // 16, 16).transpose(0, 2, 1)  # (B, 16, NV/16)
    idx_w = np.ascontiguousarray(np.tile(wrap, (1, 8, 1)))  # (B, 128, NV/16)

    featr = feat.reshape(B, C, PIX).astype(ml_dtypes.bfloat16)

    shared = {
        "rwt": rwt,
        "cw1": w1f,
        "rb": rb,
        "cb1": b1f,
        "cb2": np.full((128, 1), b2[0], dtype=np.float32),
        "ident": np.eye(128, dtype=ml_dtypes.bfloat16),
    }
    in_maps = []
    for core in range(NCORES):
        sl = slice(core * IMGS, (core + 1) * IMGS)
        m = dict(shared)
        m["feat"] = np.ascontiguousarray(featr[sl])
        m["wxd"] = np.ascontiguousarray(wxd[sl])
        m["wyd"] = np.ascontiguousarray(wyd[sl])
        m["idx"] = np.ascontiguousarray(idx_w[sl])
        in_maps.append(m)
    return in_maps, P


def kernel(**inputs):
    from concourse.bass_utils import run_bass_kernel_spmd

    in_maps, P = _host_prep(inputs)
    nc = _build(P)
    res = run_bass_kernel_spmd(nc, in_maps, list(range(NCORES)))
    out = np.empty((B, N), dtype=np.float32)
    for core in range(NCORES):
        dev = res.results[core]["out"]          # (IMGS, 128, Q), vert j at (j%128, j//128)
        full = dev.transpose(0, 2, 1).reshape(IMGS, NV)
        out[core * IMGS : (core + 1) * IMGS] = full[:, :N]
    return out


# revision 10
# speedup vs baseline: 1.6155x; 1.0060x over previous
"""nn_ContactHead Trainium2 kernel (8-core data parallel).

out = sigmoid(w2 . relu((grid_sample(feat, uv) @ reduce_w + reduce_b) @ cls_w1 + cls_b1) + cls_b2)

Everything left of the relu is linear and bilinear sampling is linear in the
features, so the channel reductions commute with the sampling; additionally the
signed w2 is folded into the combined weights with dims permuted so that
w2>=0 dims come first (P of them):
  W  = reduce_w @ cls_w1[:, perm] * w2[perm]     (1280 x 128)  [device, PE]
  bb = (reduce_b @ cls_w1 + cls_b1)[perm]*w2[perm]             [device, PE]
  logit = sum_{d<P} max(v'_d, 0) + sum_{d>=P} min(v'_d, 0) + b2
(relu(v)*w2 == max(v*w2,0) for w2>=0 and min(v*w2,0) for w2<0.)

z[d,pix] at the 1024 pixels via PE (bf16), then pre-differenced quantities
  dzx = z(x+1)-z ; dzy = z(y+1)-z ; dzxy = dzy(x+1)-dzy
  v(wx,wy) = z00 + wx*dzx + wy*(dzy + wx*dzxy)
Token rows [z00|dzy|dzx|dzxy] (1KB bf16, pixel-major) go to DRAM (PE
transpose) and come back with dma_gather (1024 int16 idx / call,
single_packet, wrapped-16 idx layout computed on host with the bilinear
weights). Blend on DVE in 2x_1P packed mode using pair-duplicated weights
(broadcast AP ends in [1,2]); the two wx multiplies (and the two follow-up
adds) are fused into double-width ops thanks to the token order. Segment
max/min on tensor_scalar (4x mode), two 2x fold-adds halve the reduce input,
tensor_reduce (1x) finishes the dot. Sigmoid+bias on ACT.

Vert layout: vert j lives at (partition j%128, column j//128).
Outputs are stored partition-major and unshuffled on host.
"""

import ml_dtypes
import numpy as np

B, C, H, W, N = 32, 1280, 32, 32, 6890
NCORES = 8
IMGS = B // NCORES          # 4 images per core
PIX = H * W                 # 1024
PPAD = 1088                 # padded pixel slots in the dims-major z tiles
NCH = C // 128              # 10 channel chunks
MID = 128
NV = 6912                   # padded verts (= 54*128)
Q = NV // 128               # 54
TOK = 512                   # token row: 4 quantities x 128 dims (bf16)
GCOLS = [8, 8, 8, 8, 8, 8, 6]          # cols per dma_gather (<=8 -> <=1024 idx)
CHUNKS = [(0, 16), (16, 16), (32, 16), (48, 6)]   # blend (col0, ncols)

_CACHE = {}


def _build(P):
    key = ("nc", P)
    if key in _CACHE:
        return _CACHE[key]

    from contextlib import ExitStack

    import concourse.bass as bass
    import concourse.tile as tile
    from concourse import bacc, mybir
    from concourse.ap import AP

    f32 = mybir.dt.float32
    bf16 = mybir.dt.bfloat16
    i16 = mybir.dt.int16
    OP = mybir.AluOpType
    ACT = mybir.ActivationFunctionType

    nc = bacc.Bacc("TRN2", target_bir_lowering=False, debug=False)

    feat_d = nc.dram_tensor("feat", [IMGS, C, PIX], bf16, kind="ExternalInput")
    wxd_d = nc.dram_tensor("wxd", [IMGS, 128, 2 * Q], bf16, kind="ExternalInput")
    wyd_d = nc.dram_tensor("wyd", [IMGS, 128, 2 * Q], bf16, kind="ExternalInput")
    idx_d = nc.dram_tensor("idx", [IMGS, 128, NV // 16], i16, kind="ExternalInput")
    rwt_d = nc.dram_tensor("rwt", [256, C], f32, kind="ExternalInput")
    cw1_d = nc.dram_tensor("cw1", [256, MID], f32, kind="ExternalInput")
    rb_d = nc.dram_tensor("rb", [256], f32, kind="ExternalInput")
    cb1_d = nc.dram_tensor("cb1", [MID], f32, kind="ExternalInput")
    cb2_d = nc.dram_tensor("cb2", [128, 1], f32, kind="ExternalInput")
    id_d = nc.dram_tensor("ident", [128, 128], bf16, kind="ExternalInput")
    ztok_d = [
        nc.dram_tensor(f"ztok{i}", [PIX, TOK], bf16) for i in range(IMGS)
    ]
    out_d = nc.dram_tensor("out", [IMGS, 128, Q], f32, kind="ExternalOutput")

    with tile.TileContext(nc) as tc, ExitStack() as ctx:
        consts = ctx.enter_context(tc.tile_pool(name="consts", bufs=1))
        prep = ctx.enter_context(tc.tile_pool(name="prep", bufs=1))
        featp = ctx.enter_context(tc.tile_pool(name="featp", bufs=2))
        zqp = ctx.enter_context(tc.tile_pool(name="zqp", bufs=8))
        gpool = ctx.enter_context(tc.tile_pool(name="gpool", bufs=2))
        tpool = ctx.enter_context(tc.tile_pool(name="tpool", bufs=2))
        irp = ctx.enter_context(tc.tile_pool(name="irp", bufs=4))
        lg = ctx.enter_context(tc.tile_pool(name="lg", bufs=2))

        # ---------------- phase 0: combined weights (PE) ----------------
        psw_ctx = ExitStack()
        psw = psw_ctx.enter_context(tc.tile_pool(name="psw", bufs=2, space="PSUM"))
        rwt_t, cw1_t = [], []
        for k in range(2):
            rt = prep.tile([128, C], f32, tag=f"rwt{k}", name=f"rwt{k}")
            nc.sync.dma_start(rt[:], rwt_d.ap()[128 * k : 128 * (k + 1), :])
            rwt_t.append(rt)
            ct = prep.tile([128, MID], f32, tag=f"cw1{k}", name=f"cw1{k}")
            nc.sync.dma_start(ct[:], cw1_d.ap()[128 * k : 128 * (k + 1), :])
            cw1_t.append(ct)

        Wt = []
        for c in range(NCH):
            pw = psw.tile([128, 128], f32, tag="pw", name=f"pw{c}")
            for k in range(2):
                nc.tensor.matmul(
                    pw[:],
                    lhsT=rwt_t[k][:, 128 * c : 128 * (c + 1)],
                    rhs=cw1_t[k][:],
                    start=(k == 0),
                    stop=(k == 1),
                )
            wt = consts.tile([128, 128], bf16, tag=f"W{c}", name=f"W{c}")
            nc.scalar.copy(wt[:], pw[:])
            Wt.append(wt)

        rb_t = prep.tile([128, 2], f32, tag="rb", name="rb")
        nc.scalar.dma_start(rb_t[:], rb_d.ap().rearrange("(k p) -> p k", p=128))
        cb1_t = prep.tile([1, MID], f32, tag="cb1", name="cb1")
        nc.scalar.dma_start(cb1_t[:], cb1_d.ap().rearrange("(one d) -> one d", one=1))
        pb = psw.tile([1, 128], f32, tag="pb", name="pb")
        for k in range(2):
            nc.tensor.matmul(
                pb[:], lhsT=rb_t[:, k : k + 1], rhs=cw1_t[k][:],
                start=(k == 0), stop=(k == 1),
            )
        brow = prep.tile([1, 128], f32, tag="brow", name="brow")
        nc.vector.tensor_tensor(out=brow[:], in0=pb[:], in1=cb1_t[:], op=OP.add)
        bbias = consts.tile([1, 128], bf16, tag="bbias", name="bbias")
        nc.scalar.copy(bbias[:], brow[:])

        ones_t = consts.tile([1, PIX], bf16, tag="ones", name="ones")
        nc.vector.memset(ones_t[:], 1.0)
        ident = consts.tile([128, 128], bf16, tag="ident", name="ident")
        nc.scalar.dma_start(ident[:], id_d.ap())
        cb2_t = consts.tile([128, 1], f32, tag="cb2", name="cb2")
        nc.scalar.dma_start(cb2_t[:], cb2_d.ap())
        psw_ctx.close()

        zps = ctx.enter_context(tc.tile_pool(name="zps", bufs=2, space="PSUM"))
        pst = ctx.enter_context(tc.tile_pool(name="pst", bufs=3, space="PSUM"))

        # host-computed bilinear weights (dup-pairs) + wrapped-16 indices,
        # loaded up front so the gather pipeline is never input-gated
        wxds, wyds, idxts = [], [], []
        for i in range(IMGS):
            wxd = irp.tile([128, 2 * Q], bf16, tag="wxd", name=f"wxd{i}")
            wyd = irp.tile([128, 2 * Q], bf16, tag="wyd", name=f"wyd{i}")
            idxt = irp.tile([128, NV // 16], i16, tag="idx", name=f"idx{i}")
            nc.scalar.dma_start(wxd[:], wxd_d.ap()[i])
            nc.scalar.dma_start(wyd[:], wyd_d.ap()[i])
            nc.scalar.dma_start(idxt[:], idx_d.ap()[i])
            wxds.append(wxd)
            wyds.append(wyd)
            idxts.append(idxt)

        for i in range(IMGS):
            wxd, wyd, idxt = wxds[i], wyds[i], idxts[i]

            # ---------------- z at pixels (PE) ----------------
            ft = featp.tile([128, NCH * PIX], bf16, tag="ft", name=f"ft{i}")
            f_i = feat_d.ap()[i]
            nc.sync.dma_start(
                ft[:],
                AP(f_i.tensor, f_i.offset,
                   [[PIX, 128], [128 * PIX, NCH], [1, PIX]]),
            )
            zp = zps.tile([128, PIX], f32, tag="zp", name=f"zp{i}")
            for ph in range(2):
                sl = slice(512 * ph, 512 * (ph + 1))
                for c in range(NCH):
                    nc.tensor.matmul(
                        zp[:, sl],
                        lhsT=Wt[c][:],
                        rhs=ft[:, PIX * c + 512 * ph : PIX * c + 512 * (ph + 1)],
                        start=(c == 0),
                        stop=False,
                        skip_group_check=True,
                    )
                nc.tensor.matmul(
                    zp[:, sl], lhsT=bbias[:], rhs=ones_t[:, sl],
                    start=False, stop=True, skip_group_check=True,
                )

            # escape + pre-differenced quantities (dims-major, bf16)
            zq = zqp.tile([128, PPAD], bf16, tag="zq", name=f"zq{i}")
            dzx = zqp.tile([128, PPAD], bf16, tag="zq", name=f"dzx{i}")
            dzy = zqp.tile([128, PPAD], bf16, tag="zq", name=f"dzy{i}")
            dzxy = zqp.tile([128, PPAD], bf16, tag="zq", name=f"dzxy{i}")
            nc.scalar.copy(zq[:, 0:PIX], zp[:])
            nc.vector.memset(zq[:, PIX:PPAD], 0.0)
            nc.vector.tensor_tensor(out=dzx[:, 0:1056], in0=zq[:, 1:1057],
                                    in1=zq[:, 0:1056], op=OP.subtract)
            nc.vector.memset(dzx[:, 1056:PPAD], 0.0)
            nc.vector.tensor_tensor(out=dzy[:, 0:1056], in0=zq[:, 32:PPAD],
                                    in1=zq[:, 0:1056], op=OP.subtract)
            nc.vector.memset(dzy[:, 1056:PPAD], 0.0)
            nc.vector.tensor_tensor(out=dzxy[:, 0:1055], in0=dzy[:, 1:1056],
                                    in1=dzy[:, 0:1055], op=OP.subtract)
            nc.vector.memset(dzxy[:, 1055:PPAD], 0.0)

            # --------- tokens [z00|dzy|dzx|dzxy] to DRAM (PE transpose) ---------
            stg = featp.tile([128, 8 * TOK], bf16, tag="stg", name=f"stg{i}")
            for b in range(8):
                pt = pst.tile([128, TOK], bf16, tag="pt", name=f"pt{i}_{b}")
                for qi, zt in enumerate((zq, dzy, dzx, dzxy)):
                    nc.tensor.transpose(
                        pt[:, 128 * qi : 128 * (qi + 1)],
                        zt[:, 128 * b : 128 * (b + 1)],
                        ident[:],
                    )
                nc.scalar.copy(stg[:, TOK * b : TOK * (b + 1)], pt[:])
            zt_i = ztok_d[i].ap()
            nc.sync.dma_start(
                AP(zt_i.tensor, zt_i.offset,
                   [[TOK, 128], [128 * TOK, 8], [1, TOK]]),
                stg[:].rearrange("p (b t) -> p b t", t=TOK),
            )

            # ---------------- gather + blend + dot ----------------
            logit = lg.tile([128, Q], f32, tag="logit", name=f"lg{i}")
            gtiles = {}
            gcol = 0
            for gi, ncols_g in enumerate(GCOLS):
                c0 = gcol
                ct0, szt = CHUNKS[[c for c, (a, s) in enumerate(CHUNKS)
                                   if a <= c0 < a + s][0]]
                if ct0 not in gtiles:
                    gtiles[ct0] = gpool.tile(
                        [128, 16 * TOK], bf16, tag="g", name=f"g{i}_{ct0}"
                    )
                gt3 = gtiles[ct0][:].rearrange("p (c t) -> p c t", t=TOK)
                nc.gpsimd.dma_gather(
                    out_ap=gt3[:, c0 - ct0 : c0 - ct0 + ncols_g, :],
                    in_ap=ztok_d[i].ap(),
                    idxs_ap=idxt[:, c0 * 8 : (c0 + ncols_g) * 8],
                    num_idxs=ncols_g * 128,
                    num_idxs_reg=ncols_g * 128,
                    elem_size=TOK,
                )
                gcol += ncols_g

            for (c0, ncl) in CHUNKS:
                gt3 = gtiles[c0][:].rearrange("p (c t) -> p c t", t=TOK)
                g3 = gt3[:, 0:ncl, :]

                def wap(wtile, npairs, c0=c0, ncl=ncl):
                    # dup-pair weight broadcast: [[p,128],[2,ncl],[0,npairs],[1,2]]
                    a = wtile[:]
                    return AP(
                        a.tensor,
                        a.offset + 2 * c0 * a.ap[-1][0],
                        [[a.ap[0][0], 128], [2 * a.ap[-1][0], ncl],
                         [0, npairs], [a.ap[-1][0], 2]],
                    )

                def pk(apv):
                    # view [..., 2n] as [..., n, 2] (same memory)
                    return apv.rearrange("p r (d2 k) -> p r d2 k", k=2)

                t1w = tpool.tile([128, ncl * 256], bf16, tag=f"t1w{ncl}",
                                 name=f"t1w_{i}_{c0}")
                t1w3 = t1w[:].rearrange("p (r d) -> p r d", d=256)
                a2w = tpool.tile([128, ncl * 256], bf16, tag=f"a2w{ncl}",
                                 name=f"a2w_{i}_{c0}")
                a2w3 = a2w[:].rearrange("p (r d) -> p r d", d=256)
                acc = tpool.tile([128, ncl * 128], bf16, tag=f"acc{ncl}",
                                 name=f"ac_{i}_{c0}")
                acc3 = acc[:].rearrange("p (r d) -> p r d", d=128)
                # [wx*dzx | wx*dzxy]  (one wide 2x op)
                nc.vector.tensor_tensor(out=pk(t1w3), in0=pk(g3[:, :, 256:512]),
                                        in1=wap(wxd, 128), op=OP.mult)
                # [z00+wx*dzx | dzy+wx*dzxy]
                nc.vector.tensor_tensor(out=a2w3, in0=g3[:, :, 0:256],
                                        in1=t1w3, op=OP.add)
                # t3 = wy * (dzy + wx*dzxy)   (reuse t1w first half)
                t3 = t1w3[:, :, 0:128]
                nc.vector.tensor_tensor(out=pk(t3), in0=pk(a2w3[:, :, 128:256]),
                                        in1=wap(wyd, 64), op=OP.mult)
                # v' = (z00+wx*dzx) + t3
                nc.vector.tensor_tensor(out=acc3, in0=a2w3[:, :, 0:128],
                                        in1=t3, op=OP.add)
                # segment rectify: max(.,0) on dims [0,P), min(.,0) on [P,128).
                # split on even boundaries so the big ops keep 4B alignment
                # and even length (2x/4x packed mode); odd leftovers run tiny.
                def rect(lo, hi, op):
                    if lo >= hi:
                        return
                    nc.vector.tensor_scalar(out=acc3[:, :, lo:hi], in0=acc3[:, :, lo:hi],
                                            scalar1=0.0, scalar2=None, op0=op)
                Pe = P & ~1
                rect(0, Pe, OP.max)
                if P & 1:
                    rect(Pe, P, OP.max)
                    rect(P, P + 1, OP.min)
                    rect(P + 1, 128, OP.min)
                else:
                    rect(P, 128, OP.min)
                # fold 128 -> 32 with 2x adds, then 1x reduce
                u64 = a2w3[:, :, 0:64]
                nc.vector.tensor_tensor(out=u64, in0=acc3[:, :, 0:64],
                                        in1=acc3[:, :, 64:128], op=OP.add)
                u32 = a2w3[:, :, 64:96]
                nc.vector.tensor_tensor(out=u32, in0=u64[:, :, 0:32],
                                        in1=u64[:, :, 32:64], op=OP.add)
                nc.vector.tensor_reduce(
                    out=logit[:, c0 : c0 + ncl].rearrange("p (r one) -> p r one", one=1),
                    in_=u32,
                    axis=mybir.AxisListType.X,
                    op=OP.add,
                )
            ostg = lg.tile([128, Q], f32, tag="ostg", name=f"os{i}")
            nc.scalar.activation(ostg[:], logit[:], ACT.Sigmoid, bias=cb2_t[:])
            nc.scalar.dma_start(out_d.ap()[i], ostg[:])

    nc.compile()
    _CACHE[key] = nc
    return nc


def _host_prep(inputs):
    feat = np.asarray(inputs["feat_map"], dtype=np.float32)
    uv = np.asarray(inputs["verts_uv"], dtype=np.float32)
    rw = np.asarray(inputs["reduce_w"], dtype=np.float32)
    rb = np.asarray(inputs["reduce_b"], dtype=np.float32)
    w1 = np.asarray(inputs["cls_w1"], dtype=np.float32)
    b1 = np.asarray(inputs["cls_b1"], dtype=np.float32)
    w2 = np.asarray(inputs["cls_w2"], dtype=np.float32)
    b2 = np.asarray(inputs["cls_b2"], dtype=np.float32)

    # fold signed w2 into cls_w1/cls_b1, dims permuted sign-sorted (P = #pos)
    perm = np.argsort(w2 < 0, kind="stable")
    P = int((w2 >= 0).sum())
    w1f = np.ascontiguousarray(w1[:, perm] * w2[perm][None, :])
    b1f = np.ascontiguousarray(b1[perm] * w2[perm])

    rwt = np.ascontiguousarray(rw.T)                      # (256, 1280)
    uvp = np.zeros((B, NV, 2), dtype=np.float32)
    uvp[:, :N, :] = uv

    # bilinear weights + base-pixel index (same clamped-floor math as device v1)
    px = (uvp[:, :, 0] + 1.0) * np.float32(15.5)
    py = (uvp[:, :, 1] + 1.0) * np.float32(15.5)
    x0 = np.clip(np.floor(px), 0.0, 30.0)
    y0 = np.clip(np.floor(py), 0.0, 30.0)
    wx = (px - x0).astype(ml_dtypes.bfloat16)             # (B, NV)
    wy = (py - y0).astype(ml_dtypes.bfloat16)
    idx = (y0 * 32 + x0).astype(np.int16)                 # (B, NV) in [0, 990]

    # vert j at (partition j%128, col j//128); dup-pair along cols
    def dup_pair(w):
        wl = w.reshape(B, Q, 128).transpose(0, 2, 1)      # (B, 128, Q)
        return np.ascontiguousarray(np.repeat(wl, 2, axis=2))   # (B, 128, 2Q)

    wxd = dup_pair(wx)
    wyd = dup_pair(wy)

    # wrapped-16 idx layout (whole image), replicated across the 8 core groups;
    # any 128-aligned window is then a contiguous col range
    wrap = idx.reshape(B, NV // 16, 16).transpose(0, 2, 1)  # (B, 16, NV/16)
    idx_w = np.ascontiguousarray(np.tile(wrap, (1, 8, 1)))  # (B, 128, NV/16)

    featr = feat.reshape(B, C, PIX).astype(ml_dtypes.bfloat16)

    shared = {
        "rwt": rwt,
        "cw1": w1f,
        "rb": rb,
        "cb1": b1f,
        "cb2": np.full((128, 1), b2[0], dtype=np.float32),
        "ident": np.eye(128, dtype=ml_dtypes.bfloat16),
    }
    in_maps = []
    for core in range(NCORES):
        sl = slice(core * IMGS, (core + 1) * IMGS)
        m = dict(shared)
        m["feat"] = np.ascontiguousarray(featr[sl])
        m["wxd"] = np.ascontiguousarray(wxd[sl])
        m["wyd"] = np.ascontiguousarray(wyd[sl])
        m["idx"] = np.ascontiguousarray(idx_w[sl])
        in_maps.append(m)
    return in_maps, P


def kernel(**inputs):
    from concourse.bass_utils import run_bass_kernel_spmd

    in_maps, P = _host_prep(inputs)
    nc = _build(P)
    res = run_bass_kernel_spmd(nc, in_maps, list(range(NCORES)))
    out = np.empty((B, N), dtype=np.float32)
    for core in range(NCORES):
        dev = res.results[core]["out"]          # (IMGS, 128, Q), vert j at (j%128, j//128)
        full = dev.transpose(0, 2, 1).reshape(IMGS, NV)
        out[core * IMGS : (core + 1) * IMGS] = full[:, :N]
    return out
